# revision 25
# baseline (speedup 1.0000x reference)
# Multi-head attention (B=2, T=2048, C=768, H=12, D=64) on 8 NeuronCores.
#
# Sharding: core i handles batch b = i // 4 and head group g = i % 4
# (3 heads each).  Host pre-transposes/casts inputs; each core computes
# q/k DIRECTLY in transposed [d, token] layout (lhsT = weight chunk,
# rhs = xT chunk), so no PE transposes are needed anywhere:
#   qT/kT[hd, tok] = sum_ci wq[ci*128:+128, hd]^T @ xT[ci, tok]
#   rope in transposed layout with host-duplicated cosT/sinT tables
#   rmsnorm via block-ones PE matmul (partition reduction) + Ln/Exp
#   scores s^T [tk,tq] = kT.T @ qT ; p = exp(s/8) (fused 2-head tiles)
#   AV: yplus += [v | 1].T @ p  (v computed in [tok, hd] layout)
#   softmax denom = ones-row of yplus; reciprocal + partition-broadcast
#   proj: out = yT.T @ wp slices -> bf16 partials, summed on host.

import numpy as np
from contextlib import ExitStack
import ml_dtypes

import concourse.hw_specs as _hw_specs
from concourse import mybir

AF = mybir.ActivationFunctionType
ALU = mybir.AluOpType

# Keep Exp/Ln in exactly one ACT table set so bacc's greedy set selection
# never bounces between table sets (each bounce is a ~1.3us table DMA).
if not getattr(_hw_specs, "_mha_act_patch", False):
    _orig_gat = _hw_specs.get_activation_tables

    def _gat_one_exp_ln_set(arch):
        tabs = _orig_gat(arch)
        for name, s in tabs.items():
            if name != "natural_log_exp_and_others":
                s.discard(AF.Exp)
                s.discard(AF.Ln)
        return tabs

    _hw_specs.get_activation_tables = _gat_one_exp_ln_set
    _hw_specs._mha_act_patch = True

import concourse.bass as bass          # noqa: E402
import concourse.tile as tile          # noqa: E402
from concourse import bacc             # noqa: E402
bacc.get_activation_tables = _hw_specs.get_activation_tables
from concourse.bass import ts          # noqa: E402
from concourse.bass_utils import run_bass_kernel_spmd  # noqa: E402

F32 = mybir.dt.float32
BF16 = mybir.dt.bfloat16
BF16NP = ml_dtypes.bfloat16

T = 2048
C = 768
HL = 3          # heads per core
D = 64
NG = HL * D     # 192, per-core qkv width
NT = T // 128   # 16 token tiles
KC = C // 128   # 6 contraction chunks
TQB = 512       # tq block
NTQ = T // TQB  # 4


def build_kernel(tc, ctx, xT, cosd, sind, wq, wk, wv, wpa, wpb, y):
    nc = tc.nc

    big = ctx.enter_context(tc.tile_pool(name="big", bufs=1))

    # ---- persistent inputs: one DMA each, already bf16/transposed ----
    xTs = big.tile([128, KC, T], BF16, tag="xTs")
    nc.sync.dma_start(out=xTs, in_=xT)
    ws = big.tile([128, KC, 3 * NG], BF16, tag="ws")
    nc.sync.dma_start(out=ws, in_=wq)   # wq dram tensor holds [wq|wk|wv]
    wqs = ws[:, :, 0:NG]
    wks = ws[:, :, NG:2 * NG]
    wvs = ws[:, :, 2 * NG:3 * NG]
    wpa_s = big.tile([128, C], BF16, tag="wpa_s")
    nc.sync.dma_start(out=wpa_s, in_=wpa)
    wpb_s = big.tile([64, C], BF16, tag="wpb_s")
    nc.sync.dma_start(out=wpb_s, in_=wpb)
    # cos/sin arrive as [32, T]; replicate to 4 row-blocks on device.
    # sin is stored SIGNED: rows j<32 = -sin (for the y2 = x2*c - x1*s
    # half after the 32-row swap), rows 32:64 = +sin; pattern repeats.
    cosd_s = big.tile([128, T], F32, tag="cosd_s")
    nc.sync.dma_start(out=cosd_s[0:32, :], in_=cosd)
    sind_s = big.tile([128, T], F32, tag="sind_s")
    nc.sync.dma_start(out=sind_s[32:64, :], in_=sind)
    nc.vector.tensor_scalar_mul(sind_s[0:32, :], sind_s[32:64, :], -1.0)
    for r in range(1, 4):
        nc.vector.tensor_copy(cosd_s[ts(r, 32), :], cosd_s[0:32, :])
    nc.vector.tensor_copy(sind_s[64:128, :], sind_s[0:64, :])

    # block-ones for the rmsnorm partition reduction; M=64-wide so the
    # reduction matmul also BROADCASTS the per-head sum to 64 rows
    onesH0 = big.tile([128, 64], BF16, tag="onesH0")
    nc.gpsimd.memset(onesH0, 0.0)
    nc.gpsimd.memset(onesH0[0:64, :], 1.0)
    onesH1 = big.tile([128, 64], BF16, tag="onesH1")
    nc.gpsimd.memset(onesH1, 0.0)
    nc.gpsimd.memset(onesH1[64:128, :], 1.0)
    onesBB = big.tile([64, 64], BF16, tag="onesBB")
    nc.gpsimd.memset(onesBB, 1.0)
    eps_c = big.tile([128, 1], F32, tag="eps_c")
    nc.gpsimd.memset(eps_c, 1.0e-6)

    # ---- persistent big tensors ----
    qT01 = big.tile([128, T], BF16, tag="qT01")
    kT01 = big.tile([128, T], BF16, tag="kT01")
    qT22 = big.tile([128, T], BF16, tag="qT22")
    kT22 = big.tile([128, T], BF16, tag="kT22")
    yTa = big.tile([128, T], BF16, tag="yTa")   # rows 0:64 head0, 64:128 h1
    yTb = big.tile([64, T], BF16, tag="yTb")    # head2
    v_all = big.tile([128, NT, HL, 65], BF16, tag="v_all")
    nc.gpsimd.memset(v_all[:, :, :, 64:65], 1.0)

    work = ctx.enter_context(tc.tile_pool(name="work", bufs=1))
    dnq = ctx.enter_context(tc.tile_pool(name="dnq", bufs=1))
    dn = ctx.enter_context(tc.tile_pool(name="dn", bufs=2))

    # ===== pass 1: qT/kT via M-packed 128-row chunks + v tiles =====
    # The fused weight [wq|wk] columns are consumed in three 128-column
    # chunks: chunk0 = q heads 0,1; chunk1 = [q head2 | k head0];
    # chunk2 = k heads 1,2.  Each chunk's 128 psum rows are two 64-row
    # head blocks that share the whole rope/rmsnorm pipeline.
    QB = 1024   # qk processing block width (2 psum banks)
    with tc.tile_pool(name="psC", bufs=2, space="PSUM") as psC, \
         tc.tile_pool(name="psM", bufs=2, space="PSUM") as psM:

        def qk_chunk(c3, blk):
            blks = ts(blk, QB)
            nm = f"c{c3}_{blk}"
            pC = psC.tile([128, QB], F32, tag="pC", name=f"pC_{nm}")
            for half in range(QB // 512):
                hs = ts(half, 512)
                bs = slice(blk * QB + half * 512,
                           blk * QB + half * 512 + 512)
                for ci in range(KC):
                    nc.tensor.matmul(pC[:, hs],
                                     lhsT=ws[:, ci, ts(c3, 128)],
                                     rhs=xTs[:, ci, bs],
                                     start=(ci == 0), stop=(ci == KC - 1))
            # rope: tc = x*cos; swapped sin-product written directly via
            # shifted-dst muls (srcs aligned, dst may shift); yr = tc+uSw
            tcC = work.tile([128, QB], F32, tag="tcC", name=f"tcC_{nm}")
            nc.vector.tensor_mul(tcC, pC, cosd_s[:, blks])
            uSw = work.tile([128, QB], F32, tag="uSw", name=f"uSw_{nm}")
            nc.vector.tensor_mul(uSw[0:32], pC[32:64], sind_s[32:64, blks])
            nc.vector.tensor_mul(uSw[32:64], pC[0:32], sind_s[0:32, blks])
            nc.vector.tensor_mul(uSw[64:96], pC[96:128],
                                 sind_s[96:128, blks])
            nc.vector.tensor_mul(uSw[96:128], pC[64:96],
                                 sind_s[64:96, blks])
            yr = work.tile([128, QB], F32, tag="yr", name=f"yr_{nm}")
            nc.vector.tensor_add(yr, tcC, uSw)
            # rmsnorm: block-ones matmul broadcasts each head's sum
            sq = work.tile([128, QB], BF16, tag="sq", name=f"sq_{nm}")
            nc.vector.tensor_mul(sq, yr, yr)
            ms = psM.tile([128, QB], F32, tag="ms", name=f"ms_{nm}")
            for half in range(QB // 512):
                hs = ts(half, 512)
                nc.tensor.matmul(ms[0:64, hs], lhsT=onesH0,
                                 rhs=sq[:, hs], start=True, stop=True)
                nc.tensor.matmul(ms[64:128, hs], lhsT=onesH1,
                                 rhs=sq[:, hs], start=True, stop=True)
            lms = dnq.tile([128, QB], F32, tag="lms", name=f"lms_{nm}")
            nc.scalar.activation(lms, ms, AF.Ln, scale=1.0 / 64.0,
                                 bias=eps_c)
            ib = dnq.tile([128, QB], F32, tag="ib", name=f"ib_{nm}")
            nc.scalar.activation(ib, lms, AF.Exp, scale=-0.5)
            # scaled bf16 writes to the packed score tensors
            if c3 == 0:
                nc.vector.tensor_mul(qT01[:, blks], yr, ib)
            elif c3 == 1:
                nc.vector.tensor_mul(qT22[0:64, blks], yr[0:64], ib[0:64])
                nc.vector.tensor_mul(qT22[64:128, blks], yr[0:64], ib[0:64])
                nc.vector.tensor_mul(kT01[0:64, blks], yr[64:128],
                                     ib[64:128])
            else:
                nc.vector.tensor_mul(kT01[64:128, blks], yr[0:64],
                                     ib[0:64])
                nc.vector.tensor_mul(kT22[0:64, blks], yr[64:128],
                                     ib[64:128])
                nc.vector.tensor_mul(kT22[64:128, blks], yr[64:128],
                                     ib[64:128])

        for blk in range(T // QB):
            for c3 in range(3):
                qk_chunk(c3, blk)

    with tc.tile_pool(name="psV", bufs=2, space="PSUM") as psV:
        for t2 in range(NT // 2):
            v_ps = psV.tile([128, 2, 512], F32, tag="v_ps",
                            name=f"v_ps_{t2}")
            for j in range(2):
                t = 2 * t2 + j
                for ci in range(KC):
                    nc.tensor.matmul(v_ps[:, j, 0:NG],
                                     lhsT=xTs[:, ci, ts(t, 128)],
                                     rhs=wvs[:, ci, :],
                                     start=(ci == 0), stop=(ci == KC - 1))
            v_ps4 = v_ps[:, :, 0:NG].rearrange("p a (h d) -> p a h d", h=HL)
            nc.vector.tensor_copy(v_all[:, 2 * t2:2 * t2 + 2, :, 0:64], v_ps4)

    # ===== pass 2: attention + projection, per tq chunk =====
    ppool = ctx.enter_context(tc.tile_pool(name="ppool", bufs=3))
    opool = ctx.enter_context(tc.tile_pool(name="opool", bufs=4))
    with tc.tile_pool(name="sps", bufs=1, space="PSUM") as sps, \
         tc.tile_pool(name="psY", bufs=1, space="PSUM") as psY:
        for tq in range(NTQ):
            tqs = ts(tq, TQB)
            yp = [psY.tile([65, TQB], F32, tag=f"yp{h}", bufs=1,
                           name=f"yp{h}_{tq}")
                  for h in range(HL)]
            for g in range(NT // 4):
                tkg = [4 * g + j for j in range(4)]
                # three 4-plane score tiles per 4-tk group, one fused exp
                # each; paired planes use opposite PE row halves.
                for tag, mm in (
                    ("sa", ((kT01, qT01, slice(0, 64), tkg[0], None, 0),
                            (kT01, qT01, slice(64, 128), tkg[0], (64, 0), 1),
                            (kT01, qT01, slice(0, 64), tkg[1], None, 0),
                            (kT01, qT01, slice(64, 128), tkg[1], (64, 0), 1))),
                    ("sb", ((kT01, qT01, slice(0, 64), tkg[2], None, 0),
                            (kT01, qT01, slice(64, 128), tkg[2], (64, 0), 1),
                            (kT01, qT01, slice(0, 64), tkg[3], None, 0),
                            (kT01, qT01, slice(64, 128), tkg[3], (64, 0), 1))),
                    ("sc", ((kT22, qT22, slice(0, 64), tkg[0], None, 2),
                            (kT22, qT22, slice(64, 128), tkg[1], (64, 0), 2),
                            (kT22, qT22, slice(0, 64), tkg[2], None, 2),
                            (kT22, qT22, slice(64, 128), tkg[3], (64, 0), 2))),
                ):
                    s = sps.tile([128, 4, TQB], F32, tag="s4",
                                 name=f"{tag}_{tq}_{g}")
                    for i, (kT, qT, half, tk, pos, _h) in enumerate(mm):
                        nc.tensor.matmul(s[:, i, :],
                                         lhsT=kT[half, ts(tk, 128)],
                                         rhs=qT[half, tqs],
                                         start=True, stop=True,
                                         tile_position=pos)
                    p = ppool.tile([128, 4, TQB], BF16, tag="p",
                                   name=f"p{tag}_{tq}_{g}")
                    nc.scalar.activation(p.rearrange("p a n -> p (a n)"),
                                         s.rearrange("p a n -> p (a n)"),
                                         AF.Exp, scale=0.125)
                    for i, (kT, qT, half, tk, pos, h) in enumerate(mm):
                        nc.tensor.matmul(yp[h], lhsT=v_all[:, tk, h, :],
                                         rhs=p[:, i, :],
                                         start=(tk == 0),
                                         stop=(tk == NT - 1))

            # normalize: row 64 of yp is the softmax denominator
            for h in range(HL):
                rec = dn.tile([1, TQB], F32, tag="rec", name=f"rec{h}_{tq}")
                nc.vector.reciprocal(rec, yp[h][64:65, :])
                rb = dn.tile([64, TQB], F32, tag="rb", name=f"rb{h}_{tq}")
                nc.gpsimd.partition_broadcast(rb, rec)
                if h == 0:
                    dst = yTa[0:64, tqs]
                elif h == 1:
                    dst = yTa[64:128, tqs]
                else:
                    dst = yTb[:, tqs]
                nc.vector.tensor_mul(dst, yp[h][0:64, :], rb)

            # projection for this tq chunk's 4 token tiles (bf16
            # partials); 2 tiles share one 4-plane psum tile (planes
            # padded to 512 f32 so each matmul dst is bank-aligned),
            # one wide copy per pair, one DMA per tq chunk.
            o_sb = opool.tile([128, 4, C], BF16, tag="o_sb",
                              name=f"o_sb_{tq}")
            for pr in range(2):
                pp = sps.tile([128, 4, 512], F32, tag="s4",
                              name=f"pp_{tq}_{pr}")
                for j in range(2):
                    t = 4 * tq + 2 * pr + j
                    for nh in range(2):
                        nc.tensor.matmul(pp[:, 2 * j + nh, 0:384],
                                         lhsT=yTa[:, ts(t, 128)],
                                         rhs=wpa_s[:, ts(nh, 384)],
                                         start=True, stop=False)
                        nc.tensor.matmul(pp[:, 2 * j + nh, 0:384],
                                         lhsT=yTb[:, ts(t, 128)],
                                         rhs=wpb_s[:, ts(nh, 384)],
                                         start=False, stop=True)
                dst = o_sb[:, 2 * pr:2 * pr + 2, :].rearrange(
                    "p a (b n) -> p a b n", b=2)
                srcv = pp[:, :, 0:384].rearrange(
                    "p (a b) n -> p a b n", b=2)
                nc.vector.tensor_copy(dst, srcv)
            nc.sync.dma_start(
                out=y[tqs, :].rearrange("(a p) n -> p a n", p=128),
                in_=o_sb)

    return (qT01, qT22, kT01, v_all, yTa, yTb, cosd_s, sind_s)


def build_nc(reps=1):
    nc = bacc.Bacc("TRN2", target_bir_lowering=False, debug=False,
                   num_devices=8)
    xT = nc.dram_tensor("xT", [128, KC, T], BF16, kind="ExternalInput").ap()
    cosd = nc.dram_tensor("cosd", [32, T], F32, kind="ExternalInput").ap()
    sind = nc.dram_tensor("sind", [32, T], F32, kind="ExternalInput").ap()
    wq = nc.dram_tensor("wq", [128, KC, 3 * NG], BF16,
                        kind="ExternalInput").ap()
    wpa = nc.dram_tensor("wpa", [128, C], BF16, kind="ExternalInput").ap()
    wpb = nc.dram_tensor("wpb", [64, C], BF16, kind="ExternalInput").ap()
    y = nc.dram_tensor("y", [T, C], BF16, kind="ExternalOutput").ap()
    with tile.TileContext(nc) as tc:
        for _ in range(reps):
            with ExitStack() as ctx:
                build_kernel(tc, ctx, xT, cosd, sind, wq, wq, wq,
                             wpa, wpb, y)
    nc.compile()
    return nc


def make_in_maps(x, cos, sin, wq, wk, wv, wproj):
    x = np.asarray(x, np.float32)
    cosd = np.ascontiguousarray(
        np.asarray(cos, np.float32).reshape(T, 32).T)      # [32, T]
    sind = np.ascontiguousarray(
        np.asarray(sin, np.float32).reshape(T, 32).T)
    wq = np.asarray(wq, np.float32)
    wk = np.asarray(wk, np.float32)
    wv = np.asarray(wv, np.float32)
    wp = np.asarray(wproj, np.float32)

    def to_pcn(w):  # [768, n] f32 -> [128, 6, n] bf16
        n = w.shape[1]
        return np.ascontiguousarray(
            w.reshape(KC, 128, n).transpose(1, 0, 2)).astype(BF16NP)

    in_maps = []
    for cid in range(8):
        b, g = divmod(cid, 4)
        sl = slice(g * NG, (g + 1) * NG)
        xTb = np.ascontiguousarray(
            x[b].T.reshape(KC, 128, T).transpose(1, 0, 2)).astype(BF16NP)
        wf = np.concatenate([wq[:, sl], wk[:, sl], wv[:, sl]], axis=1)
        in_maps.append({
            "xT": xTb,
            "cosd": cosd,
            "sind": sind,
            "wq": to_pcn(wf),
            "wpa": np.ascontiguousarray(
                wp[g * NG:g * NG + 128, :]).astype(BF16NP),
            "wpb": np.ascontiguousarray(
                wp[g * NG + 128:(g + 1) * NG, :]).astype(BF16NP),
        })
    return in_maps


_NC = None


def kernel(x, cos, sin, wq, wk, wv, wproj):
    global _NC
    if _NC is None:
        _NC = build_nc()
    in_maps = make_in_maps(x, cos, sin, wq, wk, wv, wproj)
    res = run_bass_kernel_spmd(_NC, in_maps, list(range(8)))
    outs = [r["y"].astype(np.float32) for r in res.results]
    y0 = outs[0] + outs[1] + outs[2] + outs[3]
    y1 = outs[4] + outs[5] + outs[6] + outs[7]
    return np.stack([y0, y1], axis=0).astype(np.float32)


if __name__ == "__main__":
    rng = np.random.default_rng(0)
    ins = {
        "x": rng.standard_normal((2, T, C), dtype=np.float32),
        "cos": rng.random((T, 1, 32), dtype=np.float32),
        "sin": rng.random((T, 1, 32), dtype=np.float32),
        "wq": rng.standard_normal((C, C), dtype=np.float32) / np.sqrt(C),
        "wk": rng.standard_normal((C, C), dtype=np.float32) / np.sqrt(C),
        "wv": rng.standard_normal((C, C), dtype=np.float32) / np.sqrt(C),
        "wproj": rng.standard_normal((C, C), dtype=np.float32) / np.sqrt(C),
    }
    out = kernel(**ins)
    print(out.shape, out.dtype, np.abs(out).max())
